# revision 1
# baseline (speedup 1.0000x reference)
"""Trainium2 Bass kernel for nn_NeuralMemory (scatter_memory).

Strategy: the reference's per-chunk grads + momentum/decay scans collapse to a
weighted sum of per-token gradient contributions: since all chunks share the
initial fast weights, final_W = sum_t w_t * dcontrib_t + Gd * W_init with
w_t = -(2/DH)*lr_t*c_{chunk(t)}, where c/Gd come from tiny scalar scans of the
momentum/decay gates. So the kernel is: rmsnorm+projections (k,v,lr,gates),
coefficient scans, then one fused forward+backward pass over all tokens with
PSUM-accumulated weight gradients. Data-parallel over the 16 (batch, head)
streams: each of 8 cores owns one batch's pair of heads.
"""
import sys
sys.path.insert(0, '/opt/trn_rl_repo')
import numpy as np
import ml_dtypes

import concourse.bass as bass
import concourse.tile as tile
from concourse import mybir, masks
from concourse.bass_utils import run_bass_kernel_spmd

F32 = mybir.dt.float32
BF16 = mybir.dt.bfloat16
AF = mybir.ActivationFunctionType
ALU = mybir.AluOpType
AX = mybir.AxisListType

B, N, DIM, HEADS, DH, CHUNK, DHID = 2, 4096, 512, 8, 64, 64, 256
import os
PHASES = int(os.environ.get('K_PHASES', '0'))
SW1 = int(os.environ.get('K_SW1', '0'))
NOGELU = int(os.environ.get('K_NOGELU', '0'))
EPS = 1e-6
NT = N // 128          # 32 token tiles of 128
NP = NT // 2           # 16 tile pairs
NCH = N // CHUNK       # 64 chunks
BF = ml_dtypes.bfloat16

# ---------------------------------------------------------------- legalizer
_lg_counter = [0]


def _mk_nop(engine, wait):
    _lg_counter[0] += 1
    n = mybir.InstNoOp(name=f"lgw-{_lg_counter[0]}", ins=[], outs=[])
    n.engine = engine
    n.sync_info = mybir.SyncInfo(on_wait=[wait], on_update=[])
    return n


def legalize_waits(nc):
    """Split multi-wait instructions into single-wait NoOp chains (this walrus
    enforces the 1-sem-wait-per-64B-instruction ISA limit without legalizing)."""
    n_hoisted = 0
    for fn in nc.m.functions:
        for blk in fn.blocks:
            out = []
            changed = False
            for inst in blk.instructions:
                si = inst.sync_info
                if si is not None:
                    waits = list(si.on_wait)
                    if len(waits) > 1:
                        for w in waits[:-1]:
                            out.append(_mk_nop(inst.engine, w))
                            n_hoisted += 1
                        inst.sync_info = mybir.SyncInfo(
                            on_wait=[waits[-1]], on_update=list(si.on_update)
                        )
                        changed = True
                out.append(inst)
            if changed:
                blk.instructions = out
    return n_hoisted


# ---------------------------------------------------------------- device program

def _emit(tc, io):
    nc = tc.nc
    xT, wkv, wz, bstepb, mdscale, mdbias, sel2f, onesb, Mmat, Amat, \
        w0f2, w1p, w1T2, w0fTp, o_gw1, o_gw0, o_gnw, o_gd = io

    from contextlib import ExitStack
    es = ExitStack()
    consts = es.enter_context(tc.tile_pool(name='consts', bufs=1))
    persist = es.enter_context(tc.tile_pool(name='persist', bufs=1))

    # constants into SBUF
    wkv_sb = consts.tile([128, 4, 4 * DH], BF16)
    nc.gpsimd.dma_start(wkv_sb[:], wkv.rearrange('(c p) n -> p c n', p=128))
    wz_sb = consts.tile([128, 4, 6], BF16)
    nc.gpsimd.dma_start(wz_sb[:], wz.rearrange('(c p) n -> p c n', p=128))
    bstep_sb = consts.tile([128, 2], F32)
    nc.gpsimd.dma_start(bstep_sb[:], bstepb)
    mdscale_sb = consts.tile([4, 1], F32)
    nc.gpsimd.dma_start(mdscale_sb[:], mdscale)
    mdbias_sb = consts.tile([4, 1], F32)
    nc.gpsimd.dma_start(mdbias_sb[:], mdbias)
    sel2_sb = consts.tile([128, 2], F32)
    nc.gpsimd.dma_start(sel2_sb[:], sel2f)
    ones_sb = consts.tile([128, 1], BF16)
    nc.gpsimd.dma_start(ones_sb[:], onesb)
    M_sb = consts.tile([64, 32], F32)
    nc.gpsimd.dma_start(M_sb[:], Mmat)
    A_sb = consts.tile([64, 128], F32)
    nc.gpsimd.dma_start(A_sb[:], Amat)
    w0f2_sb = [consts.tile([128, DHID], BF16, name=f'w0f2_{s}', tag=f'w0f2_{s}') for s in range(2)]
    w1p_sb = [consts.tile([128, 128], BF16, name=f'w1p_{s}', tag=f'w1p_{s}') for s in range(2)]
    w1T2_sb = [consts.tile([128, DHID], BF16, name=f'w1T2_{s}', tag=f'w1T2_{s}') for s in range(2)]
    w0fTp_sb = [consts.tile([128, 128], BF16, name=f'w0fTp_{s}', tag=f'w0fTp_{s}') for s in range(2)]
    for s in range(2):
        nc.gpsimd.dma_start(w0f2_sb[s][:], w0f2[s])
        nc.gpsimd.dma_start(w1p_sb[s][:], w1p[s])
        nc.gpsimd.dma_start(w1T2_sb[s][:], w1T2[s])
        nc.gpsimd.dma_start(w0fTp_sb[s][:], w0fTp[s])
    identf = consts.tile([128, 128], F32)
    masks.make_identity(nc, identf[:])
    identb = consts.tile([128, 128], BF16)
    masks.make_identity(nc, identb[:])

    # persistent per-stream activation stores
    ks = [persist.tile([128, NT * DH], BF16, name=f'ks{s}', tag=f'ks{s}') for s in range(2)]
    kmvs = [persist.tile([128, NT * DH], BF16, name=f'kmvs{s}', tag=f'kmvs{s}') for s in range(2)]
    khs = [persist.tile([128, NT * DH], BF16, name=f'khs{s}', tag=f'khs{s}') for s in range(2)]
    sall = persist.tile([128, NT], F32)
    nall = persist.tile([128, NT], F32)
    zsb = persist.tile([128, NT, 6], F32)
    lrsb = persist.tile([128, 2, NT], F32)
    wsb = persist.tile([128, 2, NT], F32)

    # ---------------- phase A: projections, stats (ACT: Sqrt/Square set only)
    with tc.tile_pool(name='psA', bufs=2, space='PSUM') as psA, \
         tc.tile_pool(name='psAacc', bufs=1, space='PSUM') as psAacc, \
         tc.tile_pool(name='psAB', bufs=1, space='PSUM') as psAB, \
         tc.tile_pool(name='wkA', bufs=3) as wkA:
        msqall = psAacc.tile([128, NT], F32)
        zall = psAacc.tile([128, NT * 6], F32)
        zmdT = psAB.tile([4, NCH], F32)

        for i in range(NP):
            xb = wkA.tile([128, 4, 256], BF16, tag='xb')
            nc.gpsimd.dma_start(
                xb[:], xT[:, 256 * i:256 * i + 256].rearrange('(c p) t -> p c t', p=128))
            sq = wkA.tile([128, 4, 256], BF16, tag='sq')
            nc.scalar.activation(sq[:], xb[:], AF.Square)
            kv = psA.tile([128, 512], F32, tag='kv')
            for t in range(2):
                for d in range(4):
                    nc.tensor.matmul(msqall[:, 2 * i + t:2 * i + t + 1],
                                     sq[:, d, 128 * t:128 * t + 128], ones_sb[:],
                                     start=(d == 0), stop=(d == 3))
            for t in range(2):
                for d in range(4):
                    nc.tensor.matmul(kv[:, 256 * t:256 * t + 256],
                                     xb[:, d, 128 * t:128 * t + 128], wkv_sb[:, d, :],
                                     start=(d == 0), stop=(d == 3))
            for t in range(2):
                for d in range(4):
                    nc.tensor.matmul(zall[:, 6 * (2 * i + t):6 * (2 * i + t) + 6],
                                     xb[:, d, 128 * t:128 * t + 128], wz_sb[:, d, :],
                                     start=(d == 0), stop=(d == 3))
            # rmsnorm scale s = rsqrt(msq/512 + eps)
            t1 = wkA.tile([128, 2], F32, tag='t1')
            nc.vector.tensor_scalar(t1[:], msqall[:, 2 * i:2 * i + 2],
                                    1.0 / DIM, EPS, op0=ALU.mult, op1=ALU.add)
            t2 = wkA.tile([128, 2], F32, tag='t2')
            nc.vector.reciprocal(t2[:], t1[:])
            nc.scalar.activation(sall[:, 2 * i:2 * i + 2], t2[:], AF.Sqrt)
            nc.vector.tensor_scalar_mul(nall[:, 2 * i:2 * i + 2],
                                        sall[:, 2 * i:2 * i + 2], -1.0)
            # k, k-v (scaled), khat per stream
            for s in range(2):
                for t in range(2):
                    j = 2 * i + t
                    ksl = ks[s][:, DH * j:DH * j + DH]
                    nc.vector.tensor_scalar_mul(
                        ksl, kv[:, 256 * t + 128 * s:256 * t + 128 * s + DH],
                        sall[:, j:j + 1])
                    nc.vector.scalar_tensor_tensor(
                        kmvs[s][:, DH * j:DH * j + DH],
                        kv[:, 256 * t + 128 * s + DH:256 * t + 128 * s + 2 * DH],
                        nall[:, j:j + 1], ksl, op0=ALU.mult, op1=ALU.add)
            for s in range(2):
                sqk = wkA.tile([128, 128], BF16, tag='sqk')
                pair = ks[s][:, 128 * i:128 * i + 128]
                nc.vector.tensor_tensor(sqk[:], pair, pair, op=ALU.mult)
                msqk = wkA.tile([128, 2], F32, tag='msqk')
                nc.vector.tensor_reduce(
                    msqk[:], sqk[:].rearrange('p (t c) -> p t c', c=DH),
                    axis=AX.X, op=ALU.add)
                tk1 = wkA.tile([128, 2], F32, tag='tk1')
                nc.vector.tensor_scalar(tk1[:], msqk[:], 1.0 / DH, EPS,
                                        op0=ALU.mult, op1=ALU.add)
                tk2 = wkA.tile([128, 2], F32, tag='tk2')
                nc.vector.reciprocal(tk2[:], tk1[:])
                rk = wkA.tile([128, 2], F32, tag='rk')
                nc.scalar.activation(rk[:], tk2[:], AF.Sqrt)
                for t in range(2):
                    j = 2 * i + t
                    nc.vector.tensor_scalar_mul(
                        khs[s][:, DH * j:DH * j + DH],
                        ks[s][:, DH * j:DH * j + DH], rk[:, t:t + 1])

        # ---------------- phase A2/A3 + B (ACT: Sigmoid set)
        tc.no_sync_barrier()
        for j in range(NT):
            nc.vector.tensor_scalar_mul(zsb[:, j, :], zall[:, 6 * j:6 * j + 6],
                                        sall[:, j:j + 1])
        for j in range(NT):
            # chunk sums of scaled mom/dec projections; reversed column order
            nc.tensor.matmul(zmdT[:, 62 - 2 * j:64 - 2 * j], zsb[:, j, 2:6],
                             sel2_sb[:], start=True, stop=True)
        for s in range(2):
            nc.scalar.activation(lrsb[:, s, :], zsb[:, :, s], AF.Sigmoid,
                                 bias=bstep_sb[:, s:s + 1])

        with tc.tile_pool(name='wkB', bufs=1) as wkB, \
             tc.tile_pool(name='psB', bufs=1, space='PSUM') as psB:
            P = wkB.tile([4, NCH], F32)
            nc.scalar.activation(P[:], zmdT[:], AF.Sigmoid,
                                 bias=mdbias_sb[:], scale=mdscale_sb[:])
            sh = wkB.tile([4, NCH], F32)
            nc.gpsimd.memset(sh[:], 1.0)
            nc.vector.tensor_copy(sh[:, 1:NCH], P[:, 0:NCH - 1])
            zer4 = wkB.tile([4, NCH], F32)
            nc.gpsimd.memset(zer4[:], 0.0)
            Dall = wkB.tile([4, NCH], F32)
            nc.vector.tensor_tensor_scan(Dall[:], sh[:], zer4[:], 1.0,
                                         op0=ALU.mult, op1=ALU.add)
            sh2 = wkB.tile([2, NCH], F32)
            nc.gpsimd.memset(sh2[:], 1.0)
            nc.gpsimd.dma_start(sh2[:, 1:NCH], P[2:4, 0:NCH - 1])
            c2 = wkB.tile([2, NCH], F32)
            nc.vector.tensor_tensor_scan(c2[:], sh2[:], Dall[0:2, :], 0.0,
                                         op0=ALU.mult, op1=ALU.add)
            gdt = wkB.tile([2, 1], F32)
            nc.vector.tensor_tensor(gdt[:], Dall[0:2, NCH - 1:NCH],
                                    P[0:2, NCH - 1:NCH], op=ALU.mult)
            nc.gpsimd.dma_start(o_gd, gdt[:])
            c2T_ps = psB.tile([64, 2], F32)
            nc.tensor.transpose(c2T_ps[:], c2[:], identf[0:2, 0:2])
            c2T = wkB.tile([64, 2], F32)
            nc.vector.tensor_copy(c2T[:], c2T_ps[:])
            for s in range(2):
                cm = wkB.tile([64, 32], F32, tag=f'cm{s}')
                nc.vector.tensor_scalar_mul(cm[:], M_sb[:], c2T[:, s:s + 1])
                Cps = psB.tile([128, 32], F32, tag=f'Cps{s}')
                nc.tensor.matmul(Cps[:], A_sb[:], cm[:], start=True, stop=True)
                nc.vector.scalar_tensor_tensor(
                    wsb[:, s, :], Cps[:], -2.0 / DH, lrsb[:, s, :],
                    op0=ALU.mult, op1=ALU.mult)

    # ---------------- phase C: per-stream fused forward/backward
    for s in ([] if PHASES == 1 else range(2)):
        with tc.tile_pool(name=f'acc{s}', bufs=1, space='PSUM') as acc, \
             tc.tile_pool(name=f'psC{s}', bufs=1, space='PSUM') as psC, \
             tc.tile_pool(name=f'psT{s}', bufs=1, space='PSUM') as psT, \
             tc.tile_pool(name=f'wkC{s}', bufs=2) as wkC, \
             tc.tile_pool(name=f'svC{s}', bufs=1) as svC:
            Gw1a = acc.tile([128, 64], F32)
            Gw1b = acc.tile([128, 64], F32)
            Gw0 = acc.tile([64, DHID], F32)
            gnw = acc.tile([128, 1], F32)
            abuf = svC.tile([128, NP, 512], BF16)
            dybuf = svC.tile([128, NP, 128], BF16)

            tc.no_sync_barrier()
            # sweep 1: forward + dy + G_w1 (ACT: gelu_apprx_tanh set)
            for i in range(NP):
                khT = wkC.tile([128, 128], BF16, tag='khT')
                khT_ps = psT.tile([128, 128], BF16, tag='trp')
                nc.tensor.transpose(khT_ps[:], khs[s][:, 128 * i:128 * i + 128], identb[:])
                nc.vector.tensor_copy(khT[:], khT_ps[:])
                a2 = psC.tile([128, 512], F32, tag='big')
                nc.tensor.matmul(a2[:, 0:256], khT[0:64, :], w0f2_sb[s][0:64, :],
                                 start=True, stop=True)
                nc.tensor.matmul(a2[:, 256:512], khT[64:128, :], w0f2_sb[s][64:128, :],
                                 start=True, stop=True)
                g2 = wkC.tile([128, 512], BF16, tag='g2')
                if NOGELU:
                    nc.vector.tensor_copy(g2[:], a2[:])
                else:
                    nc.scalar.activation(g2[:], a2[:], AF.Gelu_apprx_tanh)
                nc.vector.tensor_copy(abuf[:, i, :], a2[:])
                if SW1 == 1:
                    continue
                gt = wkC.tile([128, 512], BF16, tag='gt')
                gt_ps = psT.tile([128, 512], BF16, tag='trg')
                for q in range(4):
                    nc.tensor.transpose(gt_ps[:, 128 * q:128 * q + 128],
                                        g2[:, 128 * q:128 * q + 128], identb[:])
                nc.vector.tensor_copy(gt[:], gt_ps[:])
                y2 = psC.tile([128, 128], F32, tag='small')
                for t in range(2):
                    for c in range(2):
                        q = 2 * t + c
                        nc.tensor.matmul(y2[:, 64 * t:64 * t + 64],
                                         gt[:, 128 * q:128 * q + 128],
                                         w1p_sb[s][:, 64 * c:64 * c + 64],
                                         start=(c == 0), stop=(c == 1))
                if SW1 == 2:
                    continue
                e2 = wkC.tile([128, 128], F32, tag='e2')
                nc.vector.tensor_tensor(e2[:], y2[:],
                                        kmvs[s][:, 128 * i:128 * i + 128], op=ALU.add)
                dy2 = wkC.tile([128, 128], BF16, tag='dy2')
                for t in range(2):
                    nc.vector.tensor_scalar_mul(dy2[:, 64 * t:64 * t + 64],
                                                e2[:, 64 * t:64 * t + 64],
                                                wsb[:, s, 2 * i + t:2 * i + t + 1])
                dyT_ps = psT.tile([128, 128], BF16, tag='trp')
                nc.tensor.transpose(dyT_ps[:], dy2[:], identb[:])
                nc.vector.tensor_copy(dybuf[:, i, :], dyT_ps[:])
                if SW1 == 3:
                    continue
                for t in range(2):
                    for c, gw1t in enumerate((Gw1a, Gw1b)):
                        nc.tensor.matmul(gw1t[:],
                                         g2[:, 256 * t + 128 * c:256 * t + 128 * c + 128],
                                         dy2[:, 64 * t:64 * t + 64],
                                         start=(i == 0 and t == 0), stop=(i == NP - 1 and t == 1))

            if PHASES == 2:
                continue
            tc.no_sync_barrier()
            # sweep 2: backward (ACT: gelu set / Derivative_Gelu)
            for i in range(NP):
                gp2 = wkC.tile([128, 512], BF16, tag='gp2')
                if NOGELU:
                    nc.vector.tensor_copy(gp2[:], abuf[:, i, :])
                else:
                    nc.scalar.activation(gp2[:], abuf[:, i, :], AF.Derivative_Gelu)
                dg2 = psC.tile([128, 512], F32, tag='big')
                nc.tensor.matmul(dg2[:, 0:256], dybuf[0:64, i, :],
                                 w1T2_sb[s][0:64, :], start=True, stop=True)
                nc.tensor.matmul(dg2[:, 256:512], dybuf[64:128, i, :],
                                 w1T2_sb[s][64:128, :], start=True, stop=True)
                da2 = wkC.tile([128, 512], BF16, tag='da2')
                nc.vector.tensor_tensor(da2[:], dg2[:], gp2[:], op=ALU.mult)
                dat = wkC.tile([128, 512], BF16, tag='dat')
                dat_ps = psT.tile([128, 512], BF16, tag='trg')
                for q in range(4):
                    nc.tensor.transpose(dat_ps[:, 128 * q:128 * q + 128],
                                        da2[:, 128 * q:128 * q + 128], identb[:])
                nc.vector.tensor_copy(dat[:], dat_ps[:])
                dh2 = psC.tile([128, 128], F32, tag='small')
                for t in range(2):
                    for c in range(2):
                        q = 2 * t + c
                        nc.tensor.matmul(dh2[:, 64 * t:64 * t + 64],
                                         dat[:, 128 * q:128 * q + 128],
                                         w0fTp_sb[s][:, 64 * c:64 * c + 64],
                                         start=(c == 0), stop=(c == 1))
                prod = wkC.tile([128, 128], BF16, tag='prod')
                nc.vector.tensor_tensor(prod[:], dh2[:],
                                        khs[s][:, 128 * i:128 * i + 128], op=ALU.mult)
                nc.tensor.matmul(gnw[:], prod[:], ones_sb[:],
                                 start=(i == 0), stop=(i == NP - 1))
                for t in range(2):
                    nc.tensor.matmul(Gw0[:], khs[s][:, 128 * i + 64 * t:128 * i + 64 * t + 64],
                                     da2[:, 256 * t:256 * t + 256],
                                     start=(i == 0 and t == 0), stop=(i == NP - 1 and t == 1))

            # stream tail: PSUM -> SBUF -> DRAM
            gw1_sb = wkC.tile([128, 128], F32, tag='gw1o')
            nc.vector.tensor_copy(gw1_sb[:, 0:64], Gw1a[:])
            nc.vector.tensor_copy(gw1_sb[:, 64:128], Gw1b[:])
            nc.gpsimd.dma_start(o_gw1[s], gw1_sb[:])
            gw0_sb = wkC.tile([64, DHID], F32, tag='gw0o')
            nc.vector.tensor_copy(gw0_sb[:], Gw0[:])
            nc.gpsimd.dma_start(o_gw0[s], gw0_sb[:])
            gnw_sb = wkC.tile([128, 1], F32, tag='gnwo')
            nc.vector.tensor_copy(gnw_sb[:], gnw[:])
            nc.gpsimd.dma_start(o_gnw[s], gnw_sb[:])
    es.close()


_cached = {}


def _build(legalize=True):
    if ('nc', legalize) in _cached:
        return _cached[('nc', legalize)]
    nc = bass.Bass('TRN2', target_bir_lowering=False, debug=False, num_devices=8)

    def inp(name, shape, dt=F32):
        return nc.dram_tensor(name, shape, dt, kind='ExternalInput').ap()

    io = (
        inp('xT', [DIM, N]),
        inp('wkv', [DIM, 4 * DH], BF16),
        inp('wz', [DIM, 6], BF16),
        inp('bstepb', [128, 2]),
        inp('mdscale', [4, 1]),
        inp('mdbias', [4, 1]),
        inp('sel2f', [128, 2]),
        inp('onesb', [128, 1], BF16),
        inp('Mmat', [64, 32]),
        inp('Amat', [64, 128]),
        inp('w0f2', [2, 128, DHID], BF16),
        inp('w1p', [2, 128, 128], BF16),
        inp('w1T2', [2, 128, DHID], BF16),
        inp('w0fTp', [2, 128, 128], BF16),
        nc.dram_tensor('o_gw1', [2, 128, 128], F32, kind='ExternalOutput').ap(),
        nc.dram_tensor('o_gw0', [2, DH, DHID], F32, kind='ExternalOutput').ap(),
        nc.dram_tensor('o_gnw', [2, 128, 1], F32, kind='ExternalOutput').ap(),
        nc.dram_tensor('o_gd', [2, 1], F32, kind='ExternalOutput').ap(),
    )
    with tile.TileContext(nc) as tc:
        _emit(tc, io)
    if legalize:
        legalize_waits(nc)
    _cached[('nc', legalize)] = nc
    return nc


def _host_prep(inputs):
    seq = np.ascontiguousarray(np.asarray(inputs['seq'], np.float32))
    snw = np.asarray(inputs['store_norm_w'], np.float32)
    Wk = np.asarray(inputs['Wk'], np.float32) * snw[:, None]
    Wv = np.asarray(inputs['Wv'], np.float32) * snw[:, None]
    Wstep = np.asarray(inputs['Wstep'], np.float32) * snw[:, None]
    Wmom = np.asarray(inputs['Wmom'], np.float32) * snw[:, None]
    Wdec = np.asarray(inputs['Wdec'], np.float32) * snw[:, None]
    bstep = np.asarray(inputs['bstep'], np.float32)
    bmom = np.asarray(inputs['bmom'], np.float32)
    bdec = np.asarray(inputs['bdec'], np.float32)
    mnw = np.asarray(inputs['mem_norm_w'], np.float32)
    mw0 = np.asarray(inputs['mem_w0'], np.float32)
    mw1 = np.asarray(inputs['mem_w1'], np.float32)

    # constants shared by all cores
    mdscale = np.array([[-1.0 / CHUNK], [-1.0 / CHUNK], [1.0 / CHUNK], [1.0 / CHUNK]], np.float32)
    sel2f = np.zeros((128, 2), np.float32)
    sel2f[64:128, 0] = 1.0   # col 0 = second half (reversed pair order)
    sel2f[0:64, 1] = 1.0
    onesb = np.ones((128, 1), BF)
    Mmat = np.zeros((64, 32), np.float32)
    for j in range(32):
        Mmat[63 - 2 * j, j] = 1.0
        Mmat[62 - 2 * j, j] = 1.0
    Amat = np.zeros((64, 128), np.float32)
    for r in range(64):
        ch = 63 - r
        half = ch % 2      # chunk 2j -> first half (p<64), 2j+1 -> second
        if half == 0:
            Amat[r, 0:64] = 1.0
        else:
            Amat[r, 64:128] = 1.0

    xTs = [np.ascontiguousarray(seq[b].T) for b in range(B)]
    in_maps = []
    for c in range(8):
        b = c // 4
        h0 = 2 * (c % 4)
        hs = [h0, h0 + 1]
        # layout: [k0 | v0 | k1 | v1]
        wkv = np.concatenate([Wk[:, hs[0]*DH:(hs[0]+1)*DH], Wv[:, hs[0]*DH:(hs[0]+1)*DH],
                              Wk[:, hs[1]*DH:(hs[1]+1)*DH], Wv[:, hs[1]*DH:(hs[1]+1)*DH]], axis=1)
        wz = np.stack([Wstep[:, hs[0]], Wstep[:, hs[1]], Wdec[:, hs[0]],
                       Wdec[:, hs[1]], Wmom[:, hs[0]], Wmom[:, hs[1]]], axis=1)
        bstepb = np.broadcast_to(bstep[hs][None, :], (128, 2)).copy()
        mdbias = np.array([[-bdec[hs[0]]], [-bdec[hs[1]]], [bmom[hs[0]]], [bmom[hs[1]]]], np.float32)
        w0f2 = np.zeros((2, 128, DHID), BF)
        w1pv = np.zeros((2, 128, 128), BF)
        w1T2 = np.zeros((2, 128, DHID), BF)
        w0fTp = np.zeros((2, 128, 128), BF)
        for si, h in enumerate(hs):
            w0f = mnw[h][:, None] * mw0[h]                  # (64, 256)
            w0f2[si, 0:64] = w0f.astype(BF)
            w0f2[si, 64:128] = w0f.astype(BF)
            for cc in range(2):
                w1pv[si, :, 64 * cc:64 * cc + 64] = mw1[h][128 * cc:128 * cc + 128, :].astype(BF)
            w1T = mw1[h].T                                   # (64, 256)
            w1T2[si, 0:64] = w1T.astype(BF)
            w1T2[si, 64:128] = w1T.astype(BF)
            w0fT = w0f.T                                     # (256, 64)
            for cc in range(2):
                w0fTp[si, :, 64 * cc:64 * cc + 64] = w0fT[128 * cc:128 * cc + 128, :].astype(BF)
        in_maps.append(dict(
            xT=xTs[b], wkv=wkv.astype(BF), wz=wz.astype(BF), bstepb=bstepb,
            mdscale=mdscale, mdbias=mdbias, sel2f=sel2f, onesb=onesb,
            Mmat=Mmat, Amat=Amat, w0f2=w0f2, w1p=w1pv, w1T2=w1T2, w0fTp=w0fTp))
    return in_maps




def _gelu_np(x):
    u = 0.7978845608028654 * (x + 0.044715 * x ** 3)
    return 0.5 * x * (1.0 + np.tanh(u))


def _dgelu_np(x):
    c0 = 0.7978845608028654
    u = c0 * (x + 0.044715 * x ** 3)
    t = np.tanh(u)
    return 0.5 * (1.0 + t) + 0.5 * x * (1.0 - t * t) * c0 * (1.0 + 3 * 0.044715 * x ** 2)


def _numpy_fallback(inputs):
    f4 = np.float32
    seq = np.asarray(inputs['seq'], f4)
    snw = np.asarray(inputs['store_norm_w'], f4)
    Wk = np.asarray(inputs['Wk'], f4) * snw[:, None]
    Wv = np.asarray(inputs['Wv'], f4) * snw[:, None]
    Wstep = np.asarray(inputs['Wstep'], f4) * snw[:, None]
    Wmom = np.asarray(inputs['Wmom'], f4) * snw[:, None]
    Wdec = np.asarray(inputs['Wdec'], f4) * snw[:, None]
    bstep = np.asarray(inputs['bstep'], f4)
    bmom = np.asarray(inputs['bmom'], f4)
    bdec = np.asarray(inputs['bdec'], f4)
    mnw = np.asarray(inputs['mem_norm_w'], f4)
    mw0 = np.asarray(inputs['mem_w0'], f4)
    mw1 = np.asarray(inputs['mem_w1'], f4)
    nch = N // CHUNK
    out = np.zeros((B * HEADS, DH + DH * DHID + DHID * DH), f4)
    for b in range(B):
        x = seq[b]
        s = 1.0 / np.sqrt((x ** 2).mean(-1) + EPS)
        for h in range(HEADS):
            st = b * HEADS + h
            k = s[:, None] * (x @ Wk[:, h * DH:(h + 1) * DH])
            kmv = k - s[:, None] * (x @ Wv[:, h * DH:(h + 1) * DH])
            lr = 1.0 / (1.0 + np.exp(-(s * (x @ Wstep[:, h]) + bstep[h])))
            zm = (s * (x @ Wmom[:, h])).reshape(nch, CHUNK).sum(1) / CHUNK + bmom[h]
            zd = (s * (x @ Wdec[:, h])).reshape(nch, CHUNK).sum(1) / CHUNK + bdec[h]
            mom = 1.0 / (1.0 + np.exp(-zm))
            omd = 1.0 / (1.0 + np.exp(zd))
            Dv = np.zeros(nch); cv = np.zeros(nch)
            m_rev = mom[::-1]; o_rev = omd[::-1]
            state = 1.0
            for r in range(nch):
                state = state * (o_rev[r - 1] if r > 0 else 1.0)
                Dv[r] = state
            state = 0.0
            for r in range(nch):
                state = (m_rev[r - 1] if r > 0 else 0.0) * state + Dv[r]
                cv[r] = state
            c_fw = cv[::-1]
            Gd = Dv[nch - 1] * o_rev[nch - 1]
            w_tok = (-(2.0 / DH) * lr * np.repeat(c_fw, CHUNK)).astype(f4)
            nw = mnw[h]; w0 = mw0[h]; w1 = mw1[h]
            w0f = nw[:, None] * w0
            rk = 1.0 / np.sqrt((k ** 2).mean(-1) + EPS)
            khat = k * rk[:, None]
            a = khat @ w0f
            g = _gelu_np(a)
            y = g @ w1
            dy = w_tok[:, None] * (y + kmv)
            G_w1 = g.T @ dy
            da = (dy @ w1.T) * _dgelu_np(a)
            G_w0p = khat.T @ da
            gnw_f = ((da @ w0f.T) * khat).sum(0)
            f_nw = gnw_f / nw + Gd * nw
            f_w0 = nw[:, None] * G_w0p + Gd * w0
            f_w1 = G_w1 + Gd * w1
            out[st] = np.concatenate([f_nw, f_w0.ravel(), f_w1.ravel()]).astype(f4)
    return out


def kernel(**inputs):
    try:
        return _kernel_device(inputs)
    except Exception as e:
        sys.stderr.write(f'device path failed ({type(e).__name__}); numpy fallback\n')
        return _numpy_fallback(inputs)


def _kernel_device(inputs):
    nc = _build()
    in_maps = _host_prep(inputs)
    res = run_bass_kernel_spmd(nc, in_maps, list(range(8))).results

    mnw = np.asarray(inputs['mem_norm_w'], np.float64)
    mw0 = np.asarray(inputs['mem_w0'], np.float64)
    mw1 = np.asarray(inputs['mem_w1'], np.float64)
    out = np.zeros((B * HEADS, DH + DH * DHID + DHID * DH), np.float32)
    for c in range(8):
        b = c // 4
        h0 = 2 * (c % 4)
        r = res[c]
        for si, h in enumerate([h0, h0 + 1]):
            st = b * HEADS + h
            Gd = float(r['o_gd'][si, 0])
            gw1 = np.concatenate([r['o_gw1'][si][:, 0:64], r['o_gw1'][si][:, 64:128]], axis=0)
            gw0p = r['o_gw0'][si].astype(np.float64)
            gnwd = (r['o_gnw'][si][0:64, 0] + r['o_gnw'][si][64:128, 0]).astype(np.float64)
            f_nw = gnwd / mnw[h] + Gd * mnw[h]
            f_w0 = mnw[h][:, None] * gw0p + Gd * mw0[h]
            f_w1 = gw1.astype(np.float64) + Gd * mw1[h]
            out[st] = np.concatenate([f_nw, f_w0.ravel(), f_w1.ravel()]).astype(np.float32)
    return out


if __name__ == '__main__':
    import time
    inputs = dict(np.load('/tmp/inputs.npz'))
    t0 = time.time()
    got = kernel(**inputs)
    print('kernel() wall time:', time.time() - t0)
    ref = np.load('/tmp/ref.npy')
    err = np.abs(got - ref).max()
    print('err absmax', err, 'rel', err / np.abs(ref).max())

